# revision 8
# baseline (speedup 1.0000x reference)
"""KNN interaction graph (k=32, cutoff=10) on 8 Trainium2 NeuronCores.

Algorithm
---------
reference: full [N,N] masked pairwise-distance matrix + row-wise top-32.
Since ``batch`` is sorted, the masked distance matrix is block-diagonal:
row i's candidates are exactly its own molecule's atoms.  Each core owns
1024 rows (8 tiles of 128); for a 128-row tile every candidate column
lies in the window [mol_start(first row), mol_end(last row)) whose span
is bounded (~264 for these inputs), so one small matmul per tile
replaces a [128, 8192] sweep.

The matmul computes, in one K=9 fp32 contraction (accumulated in this
row order so the penalty terms cancel exactly in f32 before the small
distance terms enter):

  psum[i,j] = PEN*(s_i-s_j)^2 - 2*pos_i.pos_j + sqn_j,   s = (batch-64)/8

then v = -psum - sqn_i = -(d^2) - PEN*(ds)^2.  Cross-molecule pairs get
v <= -256 and clamp to exactly -100 (= -CUTOFF^2), matching the
reference's CUTOFF masking in the negated-square domain.  The diagonal
(v ~ 0) is the strict row max, so a 5-round max8/max_index8/
match_replace top-40 with slot 0 dropped yields the top-32 neighbours in
ascending-distance order with jax.lax.top_k's smallest-index tie-break
(max_index matches first occurrences in index order).  Weights are
sqrt(-v) on ACT (exact for f32).
"""
import os
import numpy as np

try:
    import concourse  # noqa: F401
except ImportError:
    import sys
    sys.path.insert(0, "/opt/trn_rl_repo")

N = 8192
K = 32
P = 128
NCORES = 8
TILES = 8            # 128-row tiles per core
RPC = P * TILES      # rows per core
KR = 9               # augmented contraction depth
PEN = 16384.0
NEG_BIG = -1e30
CUT2 = 100.0         # CUTOFF^2
SENT_SQN = 4.0e4     # sentinel sq-norm for window padding columns

LAST_EXEC_NS = None  # filled when BASS_KNN_TRACE=1
LAST_RESULTS = None  # full BassKernelResults when tracing

_prog_cache = {}


def _bf16_round(x):
    v = x.astype(np.float32).view(np.uint32)
    return (((v.astype(np.uint64) + 0x8000) & 0xFFFF0000).astype(np.uint32)
            .view(np.float32))


def _build_program(wmax):
    import concourse.tile as tile
    from concourse import bacc, mybir

    nc = bacc.Bacc("TRN2", target_bir_lowering=False)
    f32, i32, u32 = mybir.dt.float32, mybir.dt.int32, mybir.dt.uint32

    a_d = nc.dram_tensor("a_slab", [KR, RPC], f32, kind="ExternalInput")
    b_d = nc.dram_tensor("b_win", [KR, TILES, wmax], f32, kind="ExternalInput")
    nsq_d = nc.dram_tensor("negsqn", [P, TILES], f32, kind="ExternalInput")
    w0_d = nc.dram_tensor("w0", [P, TILES], f32, kind="ExternalInput")
    outw_d = nc.dram_tensor("outw", [TILES, P, K], f32, kind="ExternalOutput")
    outi_d = nc.dram_tensor("outi", [TILES, P, K], i32, kind="ExternalOutput")

    with tile.TileContext(nc) as tc:
        with tc.tile_pool(name="const", bufs=1) as const, \
             tc.tile_pool(name="work", bufs=3) as work, \
             tc.tile_pool(name="sel", bufs=2) as sel, \
             tc.tile_pool(name="ps", bufs=2, space="PSUM") as pp:
            a_s = const.tile([KR, RPC], f32)
            b_s = const.tile([KR, TILES, wmax], f32)
            nsq_s = const.tile([P, TILES], f32)
            w0_s = const.tile([P, TILES], f32)
            nc.gpsimd.dma_start(a_s, a_d[:, :])
            nc.gpsimd.dma_start(b_s, b_d[:, :, :])
            nc.gpsimd.dma_start(nsq_s, nsq_d[:, :])
            nc.gpsimd.dma_start(w0_s, w0_d[:, :])

            for t in range(TILES):
                psum = pp.tile([P, wmax], f32)
                nc.tensor.matmul(psum, a_s[:, t * P:(t + 1) * P], b_s[:, t, :],
                                 start=True, stop=True)
                v = work.tile([P, wmax], f32)
                nc.scalar.activation(v, psum, mybir.ActivationFunctionType.Identity,
                                     bias=nsq_s[:, t:t + 1], scale=-1.0)
                nc.vector.tensor_scalar(v, v, 0.0, -CUT2,
                                        mybir.AluOpType.min, mybir.AluOpType.max)

                vals = sel.tile([P, 40], f32, tag="vals")
                idx = sel.tile([P, 40], u32, tag="idx")
                for r in range(5):
                    mv = vals[:, r * 8:(r + 1) * 8]
                    nc.vector.max(mv, v)
                    nc.vector.max_index(idx[:, r * 8:(r + 1) * 8], mv, v)
                    if r < 4:
                        nc.vector.match_replace(v, mv, v, NEG_BIG)

                gidx_f = sel.tile([P, K], f32, tag="gidxf")
                nc.vector.tensor_scalar(gidx_f, idx[:, 1:K + 1],
                                        w0_s[:, t:t + 1], None,
                                        mybir.AluOpType.add)
                gidx = sel.tile([P, K], i32, tag="gidx")
                nc.vector.tensor_copy(gidx, gidx_f)
                wout = sel.tile([P, K], f32, tag="wout")
                nc.scalar.activation(wout, vals[:, 1:K + 1],
                                     mybir.ActivationFunctionType.Sqrt,
                                     bias=0.0, scale=-1.0)
                nc.sync.dma_start(outi_d[t, :, :], gidx)
                nc.sync.dma_start(outw_d[t, :, :], wout)
    nc.compile()
    return nc


def kernel(pos, batch):
    global LAST_EXEC_NS, LAST_RESULTS
    from concourse.bass_utils import run_bass_kernel_spmd

    pos = np.ascontiguousarray(np.asarray(pos), dtype=np.float32)
    b64 = np.asarray(batch).astype(np.int64)
    assert pos.shape == (N, 3) and b64.shape == (N,)

    x, y, z = pos[:, 0], pos[:, 1], pos[:, 2]
    sqn = ((x * x + y * y) + z * z).astype(np.float32)
    s = ((b64 - 64).astype(np.float32)) / np.float32(8.0)
    s2 = s * s
    s2h = _bf16_round(s2)
    s2l = (s2 - s2h).astype(np.float32)
    ones = np.ones(N, np.float32)

    A = np.stack([s2h, s2l, s, ones, ones, x, y, z, ones]).astype(np.float32)
    Bm = np.stack([PEN * ones, PEN * ones, np.float32(-2 * PEN) * s,
                   PEN * s2h, PEN * s2l,
                   np.float32(-2.0) * x, np.float32(-2.0) * y,
                   np.float32(-2.0) * z, sqn]).astype(np.float32)

    # per-tile candidate windows (batch is sorted)
    mol_start = np.searchsorted(b64, b64, side="left")
    mol_end = np.searchsorted(b64, b64, side="right")
    first = np.arange(0, N, P)
    w0g = mol_start[first].astype(np.int64)
    w1g = mol_end[first + P - 1].astype(np.int64)
    span = int((w1g - w0g).max())
    wmax = max(64, (span + 15) // 16 * 16)

    # pad columns with sentinels so every window is exactly wmax wide
    sent = np.zeros((KR, wmax), np.float32)
    sent[8, :] = SENT_SQN
    Bp = np.concatenate([Bm, sent], axis=1)

    in_maps = []
    for c in range(NCORES):
        r0 = c * RPC
        bwin = np.empty((KR, TILES, wmax), np.float32)
        w0c = np.empty(TILES, np.int32)
        for t in range(TILES):
            g = c * TILES + t
            w0c[t] = w0g[g]
            bwin[:, t, :] = Bp[:, w0g[g]:w0g[g] + wmax]
        in_maps.append({
            "a_slab": np.ascontiguousarray(A[:, r0:r0 + RPC]),
            "b_win": bwin,
            "negsqn": np.ascontiguousarray((-sqn[r0:r0 + RPC]).reshape(TILES, P).T),
            "w0": np.ascontiguousarray(
                np.broadcast_to(w0c[None, :].astype(np.float32), (P, TILES))),
        })

    if wmax not in _prog_cache:
        _prog_cache[wmax] = _build_program(wmax)
    nc = _prog_cache[wmax]

    trace = os.environ.get("BASS_KNN_TRACE", "") == "1"
    res = run_bass_kernel_spmd(nc, in_maps, core_ids=list(range(NCORES)),
                               trace=trace)
    LAST_EXEC_NS = res.exec_time_ns
    LAST_RESULTS = res

    iw = np.concatenate([r["outw"].reshape(RPC, K) for r in res.results])
    ii = np.concatenate([r["outi"].reshape(RPC, K) for r in res.results])
    edge_index = np.stack([ii.reshape(-1).astype(np.int32),
                           np.repeat(np.arange(N, dtype=np.int32), K)])
    edge_weight = iw.reshape(-1).astype(np.float32)
    return edge_index, edge_weight


# revision 10
# speedup vs baseline: 1.3996x; 1.3996x over previous
"""KNN interaction graph (k=32, cutoff=10) on 8 Trainium2 NeuronCores.

Algorithm
---------
reference: full [N,N] masked pairwise-distance matrix + row-wise top-32.
Since ``batch`` is sorted, the masked distance matrix is block-diagonal:
row i's candidates are exactly its own molecule's atoms.  Tiles pack
whole consecutive molecules (<=128 rows), so a tile's candidate window
is exactly its own row range: one small [<=128, W<=128] matmul per tile
replaces a [128, 8192] sweep, and the diagonal sits at window column p
for partition p (fixed -> one affine_select masks it).

One K=9 fp32 matmul computes (accumulated in this row order so the
molecule-penalty terms cancel exactly in f32 before the small distance
terms enter):

  psum[i,j] = PEN*(s_i-s_j)^2 - 2*pos_i.pos_j + sqn_j,   s = (batch-64)/8

then ACT gives v = -psum - sqn_i = -(d^2) - PEN*(ds)^2.  Cross-molecule
pairs get v <= -256 and clamp to exactly -100 (= -CUTOFF^2), matching
the reference's CUTOFF masking in the negated-square domain.  After the
diagonal is filled with -100, a 4-round max8/max_index8/match_replace
top-32 yields the neighbours in ascending-distance order with
jax.lax.top_k's smallest-index tie-break (max_index matches first
occurrences in index order).  Weights are sqrt(-v) on ACT (exact f32).

Tiles are sorted by size and snake-dealt to the 8 cores so the shared
SPMD program's per-slot window widths (max across cores) stay tight.
"""
import os
import numpy as np

try:
    import concourse  # noqa: F401
except ImportError:
    import sys
    sys.path.insert(0, "/opt/trn_rl_repo")

N = 8192
K = 32
P = 128
NCORES = 8
KR = 9               # augmented contraction depth
PEN = 16384.0
NEG_BIG = -1e30
CUT2 = 100.0         # CUTOFF^2

LAST_EXEC_NS = None  # filled when BASS_KNN_TRACE=1
LAST_RESULTS = None  # full BassKernelResults when tracing

_prog_cache = {}


def _bf16_round(x):
    v = x.astype(np.float32).view(np.uint32)
    return (((v.astype(np.uint64) + 0x8000) & 0xFFFF0000).astype(np.uint32)
            .view(np.float32))


def _build_program(widths):
    """widths: per-tile-slot window widths (same for every core)."""
    import concourse.tile as tile
    from concourse import bacc, mybir

    T = len(widths)
    wsum = int(sum(widths))
    nc = bacc.Bacc("TRN2", target_bir_lowering=False)
    f32, i32, u32 = mybir.dt.float32, mybir.dt.int32, mybir.dt.uint32

    a_d = nc.dram_tensor("a_slab", [KR, T * P], f32, kind="ExternalInput")
    b_d = nc.dram_tensor("b_win", [KR, wsum], f32, kind="ExternalInput")
    nsq_d = nc.dram_tensor("negsqn", [P, T], f32, kind="ExternalInput")
    w0_d = nc.dram_tensor("w0", [P, T], f32, kind="ExternalInput")
    outw_d = nc.dram_tensor("outw", [T, P, K], f32, kind="ExternalOutput")
    outi_d = nc.dram_tensor("outi", [T, P, K], i32, kind="ExternalOutput")

    boff = np.concatenate([[0], np.cumsum(widths)]).astype(int)

    with tile.TileContext(nc) as tc:
        with tc.tile_pool(name="const", bufs=1) as const, \
             tc.tile_pool(name="work", bufs=8) as work, \
             tc.tile_pool(name="sel", bufs=3) as sel, \
             tc.tile_pool(name="ps", bufs=8, space="PSUM") as pp:
            a_s = const.tile([KR, T * P], f32)
            b_s = const.tile([KR, wsum], f32)
            nsq_s = const.tile([P, T], f32)
            w0_s = const.tile([P, T], f32)
            nc.sync.dma_start(b_s, b_d[:, :])
            nc.sync.dma_start(a_s, a_d[:, :])
            nc.sync.dma_start(nsq_s, nsq_d[:, :])
            nc.sync.dma_start(w0_s, w0_d[:, :])

            # Phase A: matmul + psum->sbuf for every tile (PE/ACT run ahead)
            vts = []
            for t in range(T):
                W = int(widths[t])
                psum = pp.tile([P, W], f32, tag="ps")
                nc.tensor.matmul(psum, a_s[:, t * P:(t + 1) * P],
                                 b_s[:, boff[t]:boff[t] + W],
                                 start=True, stop=True)
                v = work.tile([P, W], f32, tag="v")
                nc.scalar.activation(v, psum,
                                     mybir.ActivationFunctionType.Identity,
                                     bias=nsq_s[:, t:t + 1], scale=-1.0)
                vts.append(v)

            # Phase B: mask + top-32 per tile
            for t in range(T):
                W = int(widths[t])
                v = vts[t]
                nc.gpsimd.tensor_scalar(v, v, 0.0, -CUT2,
                                        mybir.AluOpType.min,
                                        mybir.AluOpType.max)
                # diagonal: window col p == partition p -> fill exact -100
                nc.gpsimd.affine_select(v, v, [[1, W]],
                                        mybir.AluOpType.not_equal,
                                        -CUT2, base=0, channel_multiplier=-1)

                vals = sel.tile([P, K], f32, tag="vals")
                idx = sel.tile([P, K], u32, tag="idx")
                for r in range(4):
                    mv = vals[:, r * 8:(r + 1) * 8]
                    nc.vector.max(mv, v)
                    nc.vector.max_index(idx[:, r * 8:(r + 1) * 8], mv, v)
                    if r < 3:
                        nc.vector.match_replace(v, mv, v, NEG_BIG)

                gidx_f = sel.tile([P, K], f32, tag="gidxf")
                nc.gpsimd.tensor_scalar(gidx_f, idx, w0_s[:, t:t + 1], None,
                                        mybir.AluOpType.add)
                gidx = sel.tile([P, K], i32, tag="gidx")
                nc.gpsimd.tensor_copy(gidx, gidx_f)
                wout = sel.tile([P, K], f32, tag="wout")
                nc.scalar.activation(wout, vals,
                                     mybir.ActivationFunctionType.Sqrt,
                                     bias=0.0, scale=-1.0)
                nc.sync.dma_start(outi_d[t, :, :], gidx)
                nc.sync.dma_start(outw_d[t, :, :], wout)
    nc.compile()
    return nc


def _pack_tiles(batch):
    """Pack consecutive molecules into <=128-row tiles.

    Returns list of (row_start, row_cnt)."""
    sizes = np.bincount(batch)
    sizes = sizes[sizes > 0]
    if sizes.max() > P:
        raise NotImplementedError("molecule larger than 128 atoms")
    starts = np.concatenate([[0], np.cumsum(sizes)])
    tiles = []
    q = 0
    nmol = len(sizes)
    while q < nmol:
        cnt = int(sizes[q])
        q2 = q + 1
        while q2 < nmol and cnt + int(sizes[q2]) <= P:
            cnt += int(sizes[q2])
            q2 += 1
        tiles.append((int(starts[q]), cnt))
        q = q2
    return tiles


def kernel(pos, batch):
    global LAST_EXEC_NS, LAST_RESULTS
    from concourse.bass_utils import run_bass_kernel_spmd

    pos = np.ascontiguousarray(np.asarray(pos), dtype=np.float32)
    b64 = np.asarray(batch).astype(np.int64)
    assert pos.shape == (N, 3) and b64.shape == (N,)

    x, y, z = pos[:, 0], pos[:, 1], pos[:, 2]
    sqn = ((x * x + y * y) + z * z).astype(np.float32)
    s = ((b64 - 64).astype(np.float32)) / np.float32(8.0)
    s2 = s * s
    s2h = _bf16_round(s2)
    s2l = (s2 - s2h).astype(np.float32)
    ones = np.ones(N, np.float32)

    A = np.stack([s2h, s2l, s, ones, ones, x, y, z, ones]).astype(np.float32)
    Bm = np.stack([PEN * ones, PEN * ones, np.float32(-2 * PEN) * s,
                   PEN * s2h, PEN * s2l,
                   np.float32(-2.0) * x, np.float32(-2.0) * y,
                   np.float32(-2.0) * z, sqn]).astype(np.float32)
    nsq_all = (-sqn).astype(np.float32)

    # ---- tile packing and snake distribution over cores ----
    tiles = _pack_tiles(b64)
    order = sorted(range(len(tiles)), key=lambda i: -tiles[i][1])
    T = (len(tiles) + NCORES - 1) // NCORES
    core_tiles = [[] for _ in range(NCORES)]   # per core: (row_start, cnt)
    for j, oi in enumerate(order):
        grp, pos_in = divmod(j, NCORES)
        c = pos_in if grp % 2 == 0 else NCORES - 1 - pos_in
        core_tiles[c].append(tiles[oi])
    for c in range(NCORES):
        while len(core_tiles[c]) < T:
            core_tiles[c].append((0, 0))       # dummy tile

    widths = tuple(
        max(1, -(-max(core_tiles[c][t][1] for c in range(NCORES)) // 8) * 8)
        for t in range(T))
    wsum = int(sum(widths))
    boff = np.concatenate([[0], np.cumsum(widths)]).astype(int)

    in_maps = []
    for c in range(NCORES):
        a_slab = np.zeros((KR, T * P), np.float32)
        b_win = np.zeros((KR, wsum), np.float32)
        b_win[8, :] = 4.0e4   # poison: padding columns clamp to -100
        negsqn = np.zeros((P, T), np.float32)
        w0 = np.zeros((P, T), np.float32)
        for t, (r0, cnt) in enumerate(core_tiles[c]):
            if cnt == 0:
                continue
            a_slab[:, t * P:t * P + cnt] = A[:, r0:r0 + cnt]
            wid = int(widths[t])
            wend = min(r0 + wid, N)
            b_win[:, boff[t]:boff[t] + (wend - r0)] = Bm[:, r0:wend]
            negsqn[:cnt, t] = nsq_all[r0:r0 + cnt]
            w0[:, t] = np.float32(r0)
        in_maps.append({"a_slab": a_slab, "b_win": b_win,
                        "negsqn": negsqn, "w0": w0})

    if widths not in _prog_cache:
        _prog_cache[widths] = _build_program(widths)
    nc = _prog_cache[widths]

    trace = os.environ.get("BASS_KNN_TRACE", "") == "1"
    res = run_bass_kernel_spmd(nc, in_maps, core_ids=list(range(NCORES)),
                               trace=trace)
    LAST_EXEC_NS = res.exec_time_ns
    LAST_RESULTS = res

    iw = np.empty((N, K), np.float32)
    ii = np.empty((N, K), np.int32)
    for c in range(NCORES):
        ow = res.results[c]["outw"]
        oi = res.results[c]["outi"]
        for t, (r0, cnt) in enumerate(core_tiles[c]):
            if cnt == 0:
                continue
            iw[r0:r0 + cnt] = ow[t, :cnt, :]
            ii[r0:r0 + cnt] = oi[t, :cnt, :]
    edge_index = np.stack([ii.reshape(-1),
                           np.repeat(np.arange(N, dtype=np.int32), K)])
    edge_weight = iw.reshape(-1)
    return edge_index, edge_weight


# revision 11
# speedup vs baseline: 1.5447x; 1.1036x over previous
"""KNN interaction graph (k=32, cutoff=10) on 8 Trainium2 NeuronCores.

Algorithm
---------
reference: full [N,N] masked pairwise-distance matrix + row-wise top-32.
Since ``batch`` is sorted, the masked distance matrix is block-diagonal:
row i's candidates are exactly its own molecule's atoms.  Tiles pack
pairs of molecules (two-pointer best fit, <=128 rows); a tile's
candidate window is exactly its own packed rows, so one small
[<=128, W<=128] matmul per tile replaces a [128, 8192] sweep and the
diagonal sits at window column p for partition p (one compile-time
affine_select masks it).

One K=9 fp32 matmul computes (accumulated in this row order so the
molecule-penalty terms cancel exactly in f32 before the small distance
terms enter):

  psum[i,j] = PEN*(s_i-s_j)^2 - 2*pos_i.pos_j + sqn_j,   s = (batch-64)/8

then ACT gives v = -psum - sqn_i = -(d^2) - PEN*(ds)^2.  Cross-molecule
pairs (including the partner molecule packed into the same tile) get
v <= -256 and clamp to exactly -100 (= -CUTOFF^2), the reference's
CUTOFF masking in the negated-square domain.  A 4-round
max8/max_index8/match_replace top-32 then yields the neighbours in
ascending-distance order with jax.lax.top_k's smallest-index tie-break
(max_index matches first occurrences in index order).  Weights are
sqrt(-v) on ACT (exact f32).  The device returns window-local indices;
the host maps them to atom ids through each tile's column table.
"""
import os
import numpy as np

try:
    import concourse  # noqa: F401
except ImportError:
    import sys
    sys.path.insert(0, "/opt/trn_rl_repo")

N = 8192
K = 32
P = 128
NCORES = 8
KR = 9               # augmented contraction depth
PEN = 16384.0
NEG_BIG = -1e30
CUT2 = 100.0         # CUTOFF^2

LAST_EXEC_NS = None  # filled when BASS_KNN_TRACE=1
LAST_RESULTS = None  # full BassKernelResults when tracing

_prog_cache = {}


def _bf16_round(x):
    v = x.astype(np.float32).view(np.uint32)
    return (((v.astype(np.uint64) + 0x8000) & 0xFFFF0000).astype(np.uint32)
            .view(np.float32))


def _build_program(widths):
    """widths: per-tile-slot window widths (same for every core)."""
    import concourse.tile as tile
    from concourse import bacc, mybir

    T = len(widths)
    wsum = int(sum(widths))
    nc = bacc.Bacc("TRN2", target_bir_lowering=False)
    f32, u32 = mybir.dt.float32, mybir.dt.uint32

    a_d = nc.dram_tensor("a_slab", [KR, T * P], f32, kind="ExternalInput")
    b_d = nc.dram_tensor("b_win", [KR, wsum], f32, kind="ExternalInput")
    nsq_d = nc.dram_tensor("negsqn", [P, T], f32, kind="ExternalInput")
    outw_d = nc.dram_tensor("outw", [P, T, K], f32, kind="ExternalOutput")
    outi_d = nc.dram_tensor("outi", [P, T, K], u32, kind="ExternalOutput")

    boff = np.concatenate([[0], np.cumsum(widths)]).astype(int)

    with tile.TileContext(nc) as tc:
        with tc.tile_pool(name="const", bufs=1) as const, \
             tc.tile_pool(name="work", bufs=max(8, T)) as work, \
             tc.tile_pool(name="ps", bufs=8, space="PSUM") as pp:
            a_s = const.tile([KR, T * P], f32)
            b_s = const.tile([KR, wsum], f32)
            nsq_s = const.tile([P, T], f32)
            idx_all = const.tile([P, T, K], u32)
            wout_all = const.tile([P, T, K], f32)
            vals_all = const.tile([P, T, K], f32)
            nc.sync.dma_start(b_s, b_d[:, :])
            nc.sync.dma_start(a_s, a_d[:, :])
            nc.sync.dma_start(nsq_s, nsq_d[:, :])

            # Phase A: matmul + psum->sbuf (PE/ACT run ahead of everything)
            vts = []
            for t in range(T):
                W = int(widths[t])
                psum = pp.tile([P, W], f32, tag="ps")
                nc.tensor.matmul(psum, a_s[:, t * P:(t + 1) * P],
                                 b_s[:, boff[t]:boff[t] + W],
                                 start=True, stop=True)
                v = work.tile([P, W], f32, tag="v")
                nc.scalar.activation(v, psum,
                                     mybir.ActivationFunctionType.Identity,
                                     bias=nsq_s[:, t:t + 1], scale=-1.0)
                vts.append(v)

            # Phase B1: masking on GpSimd (no topk-dependent work queued
            # behind it, so it never stalls the pipeline)
            for t in range(T):
                W = int(widths[t])
                v = vts[t]
                nc.gpsimd.tensor_scalar(v, v, 0.0, -CUT2,
                                        mybir.AluOpType.min,
                                        mybir.AluOpType.max)
                nc.gpsimd.affine_select(v, v, [[1, W]],
                                        mybir.AluOpType.not_equal,
                                        -CUT2, base=0, channel_multiplier=-1)

            # Phase B2: serial top-32 chain on Vector; Sqrt trails on ACT
            for t in range(T):
                v = vts[t]
                for r in range(4):
                    mv = vals_all[:, t, r * 8:(r + 1) * 8]
                    nc.vector.max(mv, v)
                    nc.vector.max_index(idx_all[:, t, r * 8:(r + 1) * 8],
                                        mv, v)
                    if r < 3:
                        nc.vector.match_replace(v, mv, v, NEG_BIG)
                nc.scalar.activation(wout_all[:, t, :], vals_all[:, t, :],
                                     mybir.ActivationFunctionType.Sqrt,
                                     bias=0.0, scale=-1.0)

            nc.sync.dma_start(outi_d[:, :, :], idx_all)
            nc.sync.dma_start(outw_d[:, :, :], wout_all)
    nc.compile()
    return nc


def _pack_tiles(b64):
    """Two-pointer best-fit pairing of molecules into <=128-row tiles.

    Returns (tiles, sizes, starts): tiles is a list of molecule-id lists.
    """
    sizes = np.bincount(b64)
    keep = np.nonzero(sizes > 0)[0]
    sizes = sizes[keep]
    if sizes.max() > P:
        raise NotImplementedError("molecule larger than 128 atoms")
    starts = np.concatenate([[0], np.cumsum(sizes)])[:-1]
    order = np.argsort(sizes, kind="stable")
    tiles = []
    i, j = 0, len(order) - 1
    while i <= j:
        if i < j and sizes[order[i]] + sizes[order[j]] <= P:
            tiles.append([int(order[j]), int(order[i])])
            i += 1
            j -= 1
        else:
            tiles.append([int(order[j])])
            j -= 1
    return tiles, sizes, starts


def kernel(pos, batch):
    global LAST_EXEC_NS, LAST_RESULTS
    from concourse.bass_utils import run_bass_kernel_spmd

    pos = np.ascontiguousarray(np.asarray(pos), dtype=np.float32)
    b64 = np.asarray(batch).astype(np.int64)
    assert pos.shape == (N, 3) and b64.shape == (N,)

    x, y, z = pos[:, 0], pos[:, 1], pos[:, 2]
    sqn = ((x * x + y * y) + z * z).astype(np.float32)
    s = ((b64 - 64).astype(np.float32)) / np.float32(8.0)
    s2 = s * s
    s2h = _bf16_round(s2)
    s2l = (s2 - s2h).astype(np.float32)
    ones = np.ones(N, np.float32)

    A = np.stack([s2h, s2l, s, ones, ones, x, y, z, ones]).astype(np.float32)
    Bm = np.stack([PEN * ones, PEN * ones, np.float32(-2 * PEN) * s,
                   PEN * s2h, PEN * s2l,
                   np.float32(-2.0) * x, np.float32(-2.0) * y,
                   np.float32(-2.0) * z, sqn]).astype(np.float32)
    nsq_all = (-sqn).astype(np.float32)

    # ---- pack molecules into tiles, snake-deal to cores by size ----
    tiles, msizes, mstarts = _pack_tiles(b64)
    tcnt = [int(sum(msizes[q] for q in tl)) for tl in tiles]
    order = sorted(range(len(tiles)), key=lambda i: -tcnt[i])
    T = (len(tiles) + NCORES - 1) // NCORES
    core_tiles = [[] for _ in range(NCORES)]   # per core: list of tile ids
    for jj, oi in enumerate(order):
        grp, pos_in = divmod(jj, NCORES)
        c = pos_in if grp % 2 == 0 else NCORES - 1 - pos_in
        core_tiles[c].append(oi)
    for c in range(NCORES):
        while len(core_tiles[c]) < T:
            core_tiles[c].append(-1)           # dummy slot

    widths = tuple(
        max(8, -(-max((tcnt[core_tiles[c][t]] if core_tiles[c][t] >= 0 else 0)
                      for c in range(NCORES)) // 4) * 4)
        for t in range(T))
    wsum = int(sum(widths))
    boff = np.concatenate([[0], np.cumsum(widths)]).astype(int)

    # per-tile window->atom-id table (for host-side index mapping)
    wmax = int(max(widths))
    winmap = np.zeros((len(tiles), wmax), np.int64)
    rowsel = [None] * len(tiles)               # atom ids of tile rows
    for ti, tl in enumerate(tiles):
        ids = np.concatenate([np.arange(mstarts[q], mstarts[q] + msizes[q])
                              for q in tl])
        rowsel[ti] = ids
        winmap[ti, :len(ids)] = ids

    in_maps = []
    for c in range(NCORES):
        a_slab = np.zeros((KR, T * P), np.float32)
        b_win = np.zeros((KR, wsum), np.float32)
        b_win[8, :] = 4.0e4   # poison: padding columns clamp to -100
        negsqn = np.zeros((P, T), np.float32)
        for t in range(T):
            ti = core_tiles[c][t]
            if ti < 0:
                continue
            ids = rowsel[ti]
            cnt = len(ids)
            a_slab[:, t * P:t * P + cnt] = A[:, ids]
            b_win[:, boff[t]:boff[t] + cnt] = Bm[:, ids]
            negsqn[:cnt, t] = nsq_all[ids]
        in_maps.append({"a_slab": a_slab, "b_win": b_win, "negsqn": negsqn})

    if widths not in _prog_cache:
        _prog_cache[widths] = _build_program(widths)
    nc = _prog_cache[widths]

    trace = os.environ.get("BASS_KNN_TRACE", "") == "1"
    res = run_bass_kernel_spmd(nc, in_maps, core_ids=list(range(NCORES)),
                               trace=trace)
    LAST_EXEC_NS = res.exec_time_ns
    LAST_RESULTS = res

    iw = np.empty((N, K), np.float32)
    ii = np.empty((N, K), np.int64)
    for c in range(NCORES):
        ow = res.results[c]["outw"]            # [P, T, K] f32
        oi = res.results[c]["outi"]            # [P, T, K] u32
        for t in range(T):
            ti = core_tiles[c][t]
            if ti < 0:
                continue
            ids = rowsel[ti]
            cnt = len(ids)
            iw[ids] = ow[:cnt, t, :]
            ii[ids] = winmap[ti][oi[:cnt, t, :].astype(np.int64)]
    edge_index = np.stack([ii.reshape(-1).astype(np.int32),
                           np.repeat(np.arange(N, dtype=np.int32), K)])
    edge_weight = iw.reshape(-1)
    return edge_index, edge_weight


# revision 14
# speedup vs baseline: 1.5601x; 1.0100x over previous
"""KNN interaction graph (k=32, cutoff=10) on 8 Trainium2 NeuronCores.

Algorithm
---------
reference: full [N,N] masked pairwise-distance matrix + row-wise top-32.
Since ``batch`` is sorted, the masked distance matrix is block-diagonal:
row i's candidates are exactly its own molecule's atoms.  Tiles pack
pairs of molecules (two-pointer best fit, <=128 rows); a tile's
candidate window is exactly its own packed rows, so one small
[<=128, W<=128] matmul per tile replaces a [128, 8192] sweep and the
diagonal sits at window column p for partition p (one compile-time
affine_select masks it).

One K=9 fp32 matmul computes (accumulated in this row order so the
molecule-penalty terms cancel exactly in f32 before the small distance
terms enter):

  psum[i,j] = PEN*(s_i-s_j)^2 - 2*pos_i.pos_j + sqn_j,   s = (batch-64)/8

then ACT gives v = -psum - sqn_i = -(d^2) - PEN*(ds)^2.  Cross-molecule
pairs (including the partner molecule packed into the same tile) get
v <= -256 and clamp to exactly -100 (= -CUTOFF^2), the reference's
CUTOFF masking in the negated-square domain.  A 4-round
max8/max_index8/match_replace top-32 then yields the neighbours in
ascending-distance order with jax.lax.top_k's smallest-index tie-break
(max_index matches first occurrences in index order).  Weights are
sqrt(-v) on ACT (exact f32).  The device returns window-local indices;
the host maps them to atom ids through each tile's column table.
"""
import os
import numpy as np

try:
    import concourse  # noqa: F401
except ImportError:
    import sys
    sys.path.insert(0, "/opt/trn_rl_repo")

N = 8192
K = 32
P = 128
NCORES = 8
KR = 9               # augmented contraction depth
PEN = 16384.0
NEG_BIG = -1e30
CUT2 = 100.0         # CUTOFF^2

LAST_EXEC_NS = None  # filled when BASS_KNN_TRACE=1
LAST_RESULTS = None  # full BassKernelResults when tracing

_prog_cache = {}


def _bf16_round(x):
    v = x.astype(np.float32).view(np.uint32)
    return (((v.astype(np.uint64) + 0x8000) & 0xFFFF0000).astype(np.uint32)
            .view(np.float32))


def _build_program(widths):
    """widths: per-tile-slot window widths (same for every core)."""
    import concourse.tile as tile
    from concourse import bacc, mybir

    T = len(widths)
    wsum = int(sum(widths))
    nc = bacc.Bacc("TRN2", target_bir_lowering=False)
    f32, u32 = mybir.dt.float32, mybir.dt.uint32

    a_d = nc.dram_tensor("a_slab", [KR, T * P], f32, kind="ExternalInput")
    b_d = nc.dram_tensor("b_win", [KR, wsum], f32, kind="ExternalInput")
    nsq_d = nc.dram_tensor("negsqn", [P, T], f32, kind="ExternalInput")
    outw_d = nc.dram_tensor("outw", [P, T, K], f32, kind="ExternalOutput")
    outi_d = nc.dram_tensor("outi", [P, T, K], u32, kind="ExternalOutput")

    boff = np.concatenate([[0], np.cumsum(widths)]).astype(int)

    with tile.TileContext(nc) as tc:
        with tc.tile_pool(name="const", bufs=1) as const, \
             tc.tile_pool(name="work", bufs=max(8, T)) as work, \
             tc.tile_pool(name="sel", bufs=4) as sel, \
             tc.tile_pool(name="ps", bufs=8, space="PSUM") as pp:
            a_s = const.tile([KR, T * P], f32)
            b_s = const.tile([KR, wsum], f32)
            nsq_s = const.tile([P, T], f32)
            idx_all = const.tile([P, T, K], u32)
            wout_all = const.tile([P, T, K], f32)

            # ACT-table warmup: hoist the Identity/Sqrt table loads into
            # the input-DMA window instead of blocking the first tile
            wu1 = const.tile([1, 8], f32)
            wu2 = const.tile([1, 8], f32)
            nc.vector.memset(wu1, 1.0)
            nc.scalar.activation(wu2, wu1,
                                 mybir.ActivationFunctionType.Identity,
                                 bias=0.0, scale=1.0)
            nc.scalar.activation(wu2, wu1,
                                 mybir.ActivationFunctionType.Sqrt,
                                 bias=0.0, scale=1.0)

            # input DMAs on separate queues so they run in parallel
            nc.gpsimd.dma_start(b_s, b_d[:, :])
            nc.sync.dma_start(a_s, a_d[:, :])
            nc.scalar.dma_start(nsq_s, nsq_d[:, :])

            # Phase A: matmul + psum->sbuf (PE/ACT run ahead of everything)
            vts = []
            for t in range(T):
                W = int(widths[t])
                psum = pp.tile([P, W], f32, tag="ps")
                nc.tensor.matmul(psum, a_s[:, t * P:(t + 1) * P],
                                 b_s[:, boff[t]:boff[t] + W],
                                 start=True, stop=True)
                v = work.tile([P, W], f32, tag="v")
                nc.scalar.activation(v, psum,
                                     mybir.ActivationFunctionType.Identity,
                                     bias=nsq_s[:, t:t + 1], scale=-1.0)
                vts.append(v)

            # Phase B1: masking on GpSimd (no topk-dependent work queued
            # behind it, so it never stalls the pipeline)
            for t in range(T):
                W = int(widths[t])
                v = vts[t]
                nc.gpsimd.tensor_scalar(v, v, 0.0, -CUT2,
                                        mybir.AluOpType.min,
                                        mybir.AluOpType.max)
                nc.gpsimd.affine_select(v, v, [[1, W]],
                                        mybir.AluOpType.not_equal,
                                        -CUT2, base=0, channel_multiplier=-1)

            # Phase B2: serial top-32 chain on Vector; Sqrt trails on ACT
            T2 = T // 2
            for t in range(T):
                v = vts[t]
                vals = sel.tile([P, K], f32, tag="vals")
                for r in range(4):
                    mv = vals[:, r * 8:(r + 1) * 8]
                    nc.vector.max(mv, v)
                    nc.vector.max_index(idx_all[:, t, r * 8:(r + 1) * 8],
                                        mv, v)
                    if r < 3:
                        nc.vector.match_replace(v, mv, v, NEG_BIG)
                nc.scalar.activation(wout_all[:, t, :], vals,
                                     mybir.ActivationFunctionType.Sqrt,
                                     bias=0.0, scale=-1.0)
                if t == T2 - 1:
                    # first-half outputs overlap the second half's topk
                    nc.sync.dma_start(outi_d[:, :T2, :], idx_all[:, :T2, :])
                    nc.sync.dma_start(outw_d[:, :T2, :], wout_all[:, :T2, :])

            nc.sync.dma_start(outi_d[:, T2:, :], idx_all[:, T2:, :])
            nc.sync.dma_start(outw_d[:, T2:, :], wout_all[:, T2:, :])
    nc.compile()
    return nc


def _pack_tiles(b64):
    """Two-pointer best-fit pairing of molecules into <=128-row tiles.

    Returns (tiles, sizes, starts): tiles is a list of molecule-id lists.
    """
    sizes = np.bincount(b64)
    keep = np.nonzero(sizes > 0)[0]
    sizes = sizes[keep]
    if sizes.max() > P:
        raise NotImplementedError("molecule larger than 128 atoms")
    starts = np.concatenate([[0], np.cumsum(sizes)])[:-1]
    order = np.argsort(sizes, kind="stable")
    tiles = []
    i, j = 0, len(order) - 1
    while i <= j:
        if i < j and sizes[order[i]] + sizes[order[j]] <= P:
            tiles.append([int(order[j]), int(order[i])])
            i += 1
            j -= 1
        else:
            tiles.append([int(order[j])])
            j -= 1
    return tiles, sizes, starts


def kernel(pos, batch):
    global LAST_EXEC_NS, LAST_RESULTS
    from concourse.bass_utils import run_bass_kernel_spmd

    pos = np.ascontiguousarray(np.asarray(pos), dtype=np.float32)
    b64 = np.asarray(batch).astype(np.int64)
    assert pos.shape == (N, 3) and b64.shape == (N,)

    x, y, z = pos[:, 0], pos[:, 1], pos[:, 2]
    sqn = ((x * x + y * y) + z * z).astype(np.float32)
    s = ((b64 - 64).astype(np.float32)) / np.float32(8.0)
    s2 = s * s
    s2h = _bf16_round(s2)
    s2l = (s2 - s2h).astype(np.float32)
    ones = np.ones(N, np.float32)

    A = np.stack([s2h, s2l, s, ones, ones, x, y, z, ones]).astype(np.float32)
    Bm = np.stack([PEN * ones, PEN * ones, np.float32(-2 * PEN) * s,
                   PEN * s2h, PEN * s2l,
                   np.float32(-2.0) * x, np.float32(-2.0) * y,
                   np.float32(-2.0) * z, sqn]).astype(np.float32)
    nsq_all = (-sqn).astype(np.float32)

    # ---- pack molecules into tiles, snake-deal to cores by size ----
    tiles, msizes, mstarts = _pack_tiles(b64)
    tcnt = [int(sum(msizes[q] for q in tl)) for tl in tiles]
    order = sorted(range(len(tiles)), key=lambda i: -tcnt[i])
    T = (len(tiles) + NCORES - 1) // NCORES
    core_tiles = [[] for _ in range(NCORES)]   # per core: list of tile ids
    for jj, oi in enumerate(order):
        grp, pos_in = divmod(jj, NCORES)
        c = pos_in if grp % 2 == 0 else NCORES - 1 - pos_in
        core_tiles[c].append(oi)
    for c in range(NCORES):
        while len(core_tiles[c]) < T:
            core_tiles[c].append(-1)           # dummy slot

    widths = tuple(
        max(8, -(-max((tcnt[core_tiles[c][t]] if core_tiles[c][t] >= 0 else 0)
                      for c in range(NCORES)) // 4) * 4)
        for t in range(T))
    wsum = int(sum(widths))
    boff = np.concatenate([[0], np.cumsum(widths)]).astype(int)

    # per-tile window->atom-id table (for host-side index mapping)
    wmax = int(max(widths))
    winmap = np.zeros((len(tiles), wmax), np.int64)
    rowsel = [None] * len(tiles)               # atom ids of tile rows
    for ti, tl in enumerate(tiles):
        ids = np.concatenate([np.arange(mstarts[q], mstarts[q] + msizes[q])
                              for q in tl])
        rowsel[ti] = ids
        winmap[ti, :len(ids)] = ids

    in_maps = []
    for c in range(NCORES):
        a_slab = np.zeros((KR, T * P), np.float32)
        b_win = np.zeros((KR, wsum), np.float32)
        b_win[8, :] = 4.0e4   # poison: padding columns clamp to -100
        negsqn = np.zeros((P, T), np.float32)
        for t in range(T):
            ti = core_tiles[c][t]
            if ti < 0:
                continue
            ids = rowsel[ti]
            cnt = len(ids)
            a_slab[:, t * P:t * P + cnt] = A[:, ids]
            b_win[:, boff[t]:boff[t] + cnt] = Bm[:, ids]
            negsqn[:cnt, t] = nsq_all[ids]
        in_maps.append({"a_slab": a_slab, "b_win": b_win, "negsqn": negsqn})

    if widths not in _prog_cache:
        _prog_cache[widths] = _build_program(widths)
    nc = _prog_cache[widths]

    trace = os.environ.get("BASS_KNN_TRACE", "") == "1"
    res = run_bass_kernel_spmd(nc, in_maps, core_ids=list(range(NCORES)),
                               trace=trace)
    LAST_EXEC_NS = res.exec_time_ns
    LAST_RESULTS = res

    iw = np.empty((N, K), np.float32)
    ii = np.empty((N, K), np.int64)
    for c in range(NCORES):
        ow = res.results[c]["outw"]            # [P, T, K] f32
        oi = res.results[c]["outi"]            # [P, T, K] u32
        for t in range(T):
            ti = core_tiles[c][t]
            if ti < 0:
                continue
            ids = rowsel[ti]
            cnt = len(ids)
            iw[ids] = ow[:cnt, t, :]
            ii[ids] = winmap[ti][oi[:cnt, t, :].astype(np.int64)]
    edge_index = np.stack([ii.reshape(-1).astype(np.int32),
                           np.repeat(np.arange(N, dtype=np.int32), K)])
    edge_weight = iw.reshape(-1)
    return edge_index, edge_weight
